# revision 12
# baseline (speedup 1.0000x reference)
"""CrossAttention Trainium2 kernel — head-pair interleaved attention.

Full inputs in, full output out. Data-parallel over batch: core b computes
batch item b of 8.

Layouts as in v2a (everything transposed so the PE contraction dim is the
partition dim). Phase 2 processes HEAD PAIRS: the S matmuls for heads 2p
and 2p+1 alternate between PE row-halves (partitions 0-63 / 64-127), which
the PE array streams concurrently (~2x S throughput measured). The exp
activations on the scalar engine are the phase-2 floor; pt-multiplies are
split across vector and gpsimd, and half the output projection (d-chunks
0-2) is interleaved into pairs 3-5 to shorten the tail.

Rowsums are reshaped 1x1024 -> 4x256 via small DMAs so reciprocals use
more DVE lanes (1.7us instead of 6.5us per batch).
"""

import numpy as np

B, L, DIM, H, HD = 8, 1024, 768, 12, 64
NCORES = 8
CP = DIM // 128  # 6 chunks of the contraction/feature dim
KC = L // 128    # 8 k-chunks
SCALE = HD ** -0.5
LN_OFF = float(np.log(256.0))

_CACHE = {}


def _build():
    import concourse.bass as bass
    import concourse.mybir as mybir
    import concourse.tile as tile
    from concourse import bacc

    f32 = mybir.dt.float32
    f16 = mybir.dt.float16
    AF = mybir.ActivationFunctionType

    nc = bacc.Bacc("TRN2", target_bir_lowering=False, debug=False)

    qT = nc.dram_tensor("qT", [DIM, L], f16, kind="ExternalInput")
    kvT = nc.dram_tensor("kvT", [DIM, L], f16, kind="ExternalInput")
    wq = nc.dram_tensor("wq", [DIM, DIM], f16, kind="ExternalInput")  # [c, d]
    wk = nc.dram_tensor("wk", [DIM, DIM], f16, kind="ExternalInput")  # [c, d]
    wv = nc.dram_tensor("wv", [DIM, DIM], f16, kind="ExternalInput")  # [c, d]
    wp = nc.dram_tensor("wp", [DIM, DIM], f16, kind="ExternalInput")  # [d, e]
    bias = nc.dram_tensor("bias", [128, DIM], f32, kind="ExternalInput")
    epos = nc.dram_tensor("epos", [H, L, L], f16, kind="ExternalInput")  # [h,k,q]
    out = nc.dram_tensor("out", [L, DIM], f32, kind="ExternalOutput")
    rscr = nc.dram_tensor("rs_scratch", [72, 256], f32)

    # which (pair, k) slots run the first-half (d=0..2) out-projection for
    # which q-chunk; all after heads 0-5 are normalized at end of pair 2
    OPROJ_SLOTS = {
        (3, 2): 0, (3, 4): 1, (3, 6): 2,
        (4, 2): 3, (4, 4): 4, (4, 6): 5,
        (5, 2): 6, (5, 4): 7,
    }

    with tile.TileContext(nc) as tc:
        with tc.tile_pool(name="persist", bufs=1) as persist:
            QT = persist.tile([128, CP, L], f16)   # pair p: heads 2p, 2p+1
            KT = persist.tile([128, CP, L], f16)
            Vt = [
                persist.tile([128, H, HD + 1], f16, name=f"Vt{k}", tag=f"V{k}")
                for k in range(KC)
            ]
            wp_sb = persist.tile([128, CP, DIM], f16)
            bias_bc = persist.tile([128, DIM], f32)
            rs = persist.tile([72, 256], f32)      # 4 rows per head, 32-aligned batches
            recip = persist.tile([72, 256], f32)

            nc.sync.dma_start(wp_sb[:], wp.rearrange("(a p) d -> p a d", p=128))
            nc.sync.dma_start(bias_bc[:], bias[:])
            expb = persist.tile([128, 1], f32)
            nc.vector.memset(expb[:], -LN_OFF)

            # ---------------- phase 1: projections ----------------
            with (
                tc.tile_pool(name="ph1", bufs=1) as ph1,
                tc.tile_pool(name="psA", bufs=2, space="PSUM") as psA,
            ):
                q_c = [ph1.tile([128, L], f16, name=f"qc{c}") for c in range(CP)]
                kv_c = [ph1.tile([128, L], f16, name=f"kvc{c}") for c in range(CP)]
                wq_c = [ph1.tile([128, DIM], f16, name=f"wqc{c}") for c in range(CP)]
                wk_c = [ph1.tile([128, DIM], f16, name=f"wkc{c}") for c in range(CP)]
                wv_c = [ph1.tile([128, DIM], f16, name=f"wvc{c}") for c in range(CP)]
                for c in range(CP):
                    r = slice(c * 128, (c + 1) * 128)
                    nc.sync.dma_start(wq_c[c][:], wq[r, :])
                    nc.sync.dma_start(q_c[c][:], qT[r, :])
                for c in range(CP):
                    r = slice(c * 128, (c + 1) * 128)
                    nc.sync.dma_start(wk_c[c][:], wk[r, :])
                    nc.sync.dma_start(kv_c[c][:], kvT[r, :])
                for c in range(CP):
                    r = slice(c * 128, (c + 1) * 128)
                    nc.sync.dma_start(wv_c[c][:], wv[r, :])

                for w_c, x_c, dst in ((wq_c, q_c, QT), (wk_c, kv_c, KT)):
                    for pp in range(CP // 2):
                        psL = psA.tile([128, L], f32, tag="proj", name=f"psL{pp}")
                        psR = psA.tile([128, L], f32, tag="proj", name=f"psR{pp}")
                        for c in range(CP):
                            for pi, ps in ((0, psL), (1, psR)):
                                p = pp * 2 + pi
                                for hf in range(2):
                                    nc.tensor.matmul(
                                        ps[:, hf * 512:(hf + 1) * 512],
                                        w_c[c][:, p * 128:(p + 1) * 128],
                                        x_c[c][:, hf * 512:(hf + 1) * 512],
                                        start=(c == 0),
                                        stop=(c == CP - 1),
                                    )
                        nc.vector.tensor_copy(dst[:, pp * 2, :], psL[:])
                        nc.vector.tensor_copy(dst[:, pp * 2 + 1, :], psR[:])

                for k in range(KC):
                    ps = psA.tile([128, DIM], f32, tag="proj")
                    for c in range(CP):
                        for lo, sz in ((0, 512), (512, 256)):
                            nc.tensor.matmul(
                                ps[:, lo:lo + sz],
                                kv_c[c][:, k * 128:(k + 1) * 128],
                                wv_c[c][:, lo:lo + sz],
                                start=(c == 0),
                                stop=(c == CP - 1),
                            )
                    nc.vector.memset(Vt[k][:, :, HD:HD + 1], 1.0)
                    nc.scalar.copy(
                        Vt[k][:, :, 0:HD],
                        ps[:].rearrange("p (h d) -> p h d", d=HD),
                    )

            # ---------------- phase 2: attention (head pairs) -------------
            with (
                tc.tile_pool(name="xt", bufs=1) as xtp,
                tc.tile_pool(name="oaccp", bufs=8) as oaccp,
            ):
              with (
                tc.tile_pool(name="eposp", bufs=10) as eposp,
                tc.tile_pool(name="praw", bufs=4) as praw,
                tc.tile_pool(name="ptp", bufs=4) as ptp,
                tc.tile_pool(name="xtup", bufs=6) as xtup,
                tc.tile_pool(name="bcp", bufs=2) as bcp,
                tc.tile_pool(name="psS", bufs=2, space="PSUM") as psS,
                tc.tile_pool(name="psO", bufs=2, space="PSUM") as psO,
              ):
                XT = xtp.tile([128, CP, L], f16)
                xtu = [None] * H
                oacc = [None] * KC

                def rrow(h):
                    # 32-aligned batches: DVE ops must start at partition 0/32/64
                    if h < 6:
                        return h * 4
                    if h < 10:
                        return 32 + (h - 6) * 4
                    return 64 + (h - 10) * 4

                def gather_rowsum(h):
                    # xtu[h][64:65, :] (1x1024) -> rs 4x256 rows
                    r0 = rrow(h)
                    for s in range(4):
                        nc.sync.dma_start(
                            rs[r0 + s:r0 + s + 1, :],
                            xtu[h][64:65, s * 256:(s + 1) * 256],
                        )

                def normalize(h, eng):
                    p, sub = divmod(h, 2)
                    bc = bcp.tile([64, L], f32, name=f"bc{h}", tag="bc")
                    r0 = rrow(h)
                    for s in range(4):
                        nc.sync.dma_start(
                            bc[:, s * 256:(s + 1) * 256],
                            rscr[r0 + s:r0 + s + 1, :].broadcast_to([64, 256]),
                        )
                    eng.tensor_mul(
                        XT[sub * 64:(sub + 1) * 64, p, :],
                        xtu[h][0:64, :],
                        bc[:],
                    )

                for p in range(CP):
                    hA, hB = 2 * p, 2 * p + 1
                    o_psA = psO.tile([65, L], f32, tag="o", name=f"oA{p}")
                    o_psB = psO.tile([65, L], f32, tag="o", name=f"oB{p}")
                    for k in range(KC):
                        kblk = slice(k * 128, (k + 1) * 128)
                        s_psA = psS.tile([128, L], f32, tag="s", name=f"sA{p}_{k}")
                        s_psB = psS.tile([128, L], f32, tag="s", name=f"sB{p}_{k}")
                        for hf in range(2):
                            hblk = slice(hf * 512, (hf + 1) * 512)
                            nc.tensor.matmul(
                                s_psA[:, hblk], KT[0:64, p, kblk], QT[0:64, p, hblk],
                            )
                            nc.tensor.matmul(
                                s_psB[:, hblk], KT[64:128, p, kblk], QT[64:128, p, hblk],
                            )
                        prA = praw.tile([128, L], f16, tag="pr")
                        nc.scalar.activation(prA[:], s_psA[:], AF.Exp, bias=expb[:])
                        epA = eposp.tile([128, L], f16, tag="ep")
                        nc.sync.dma_start(epA[:], epos[hA, kblk, :])
                        ptA = ptp.tile([128, L], f16, tag="pt")
                        nc.vector.tensor_mul(ptA[:], prA[:], epA[:])

                        prB = praw.tile([128, L], f16, tag="pr")
                        nc.scalar.activation(prB[:], s_psB[:], AF.Exp, bias=expb[:])
                        epB = eposp.tile([128, L], f16, tag="ep")
                        nc.sync.dma_start(epB[:], epos[hB, kblk, :])
                        ptB = ptp.tile([128, L], f16, tag="pt")
                        if k % 2 == 0:
                            nc.gpsimd.tensor_mul(ptB[:], prB[:], epB[:])
                        else:
                            nc.vector.tensor_mul(ptB[:], prB[:], epB[:])

                        for hf in range(2):
                            hblk = slice(hf * 512, (hf + 1) * 512)
                            nc.tensor.matmul(
                                o_psA[:, hblk], Vt[k][:, hA, :], ptA[:, hblk],
                                start=(k == 0), stop=(k == KC - 1),
                            )
                            nc.tensor.matmul(
                                o_psB[:, hblk], Vt[k][:, hB, :], ptB[:, hblk],
                                start=(k == 0), stop=(k == KC - 1),
                            )

                        qc = OPROJ_SLOTS.get((p, k))
                        if qc is not None:
                            po = psS.tile([128, DIM], f32, tag="s", name=f"po{qc}")
                            for d in range(CP // 2):
                                for lo, sz in ((0, 512), (512, 256)):
                                    nc.tensor.matmul(
                                        po[:, lo:lo + sz],
                                        XT[:, d, qc * 128:(qc + 1) * 128],
                                        wp_sb[:, d, lo:lo + sz],
                                        start=(d == 0),
                                        stop=(d == CP // 2 - 1),
                                    )
                            oacc[qc] = oaccp.tile(
                                [128, DIM], f32, tag="oacc", name=f"oacc{qc}"
                            )
                            nc.vector.tensor_add(oacc[qc][:], po[:], bias_bc[:])

                    for h, o_ps in ((hA, o_psA), (hB, o_psB)):
                        xtu[h] = xtup.tile([65, L], f32, name=f"xtu{h}", tag="xtu")
                        nc.vector.tensor_copy(xtu[h][:], o_ps[:])
                        gather_rowsum(h)

                    if p == 2:
                        nc.vector.reciprocal(recip[0:24, :], rs[0:24, :])
                        nc.sync.dma_start(rscr[0:24, :], recip[0:24, :])
                        for hh in range(6):
                            normalize(hh, nc.vector if hh % 2 else nc.gpsimd)
                    if p == 4:
                        nc.vector.reciprocal(recip[32:48, :], rs[32:48, :])
                        nc.sync.dma_start(rscr[32:48, :], recip[32:48, :])
                        for hh in range(6, 10):
                            normalize(hh, nc.vector if hh % 2 else nc.gpsimd)
                    if p == 5:
                        nc.vector.reciprocal(recip[64:72, :], rs[64:72, :])
                        nc.sync.dma_start(rscr[64:72, :], recip[64:72, :])
                        for hh in range(10, 12):
                            normalize(hh, nc.vector if hh % 2 else nc.gpsimd)

              # ---------------- phase 3: second-half output projection ----
              with (
                  tc.tile_pool(name="outp", bufs=3) as outp,
                  tc.tile_pool(name="psOut", bufs=3, space="PSUM") as psOut,
              ):
                  for qc in range(KC):
                      ps = psOut.tile([128, DIM], f32)
                      for d in range(CP // 2, CP):
                          for lo, sz in ((0, 512), (512, 256)):
                              nc.tensor.matmul(
                                  ps[:, lo:lo + sz],
                                  XT[:, d, qc * 128:(qc + 1) * 128],
                                  wp_sb[:, d, lo:lo + sz],
                                  start=(d == CP // 2),
                                  stop=(d == CP - 1),
                              )
                      ot = outp.tile([128, DIM], f32)
                      nc.vector.tensor_add(ot[:], ps[:], oacc[qc][:])
                      nc.sync.dma_start(out[qc * 128:(qc + 1) * 128, :], ot[:])

    nc.compile()
    return nc


def _get_nc():
    if "nc" not in _CACHE:
        _CACHE["nc"] = _build()
    return _CACHE["nc"]


def _host_prep(q, kv, attn_pos, Wq, Wkv, Wproj, bproj):
    q = np.asarray(q, dtype=np.float32)
    kv = np.asarray(kv, dtype=np.float32)
    attn_pos = np.asarray(attn_pos, dtype=np.float32)
    Wq = np.asarray(Wq, dtype=np.float32)
    Wkv = np.asarray(Wkv, dtype=np.float32)
    Wproj = np.asarray(Wproj, dtype=np.float32)
    bproj = np.asarray(bproj, dtype=np.float32)

    wq = np.ascontiguousarray((Wq * SCALE).T).astype(np.float16)   # [c, d]
    wk = np.ascontiguousarray(Wkv[:DIM].T).astype(np.float16)      # [c, d]
    wv = np.ascontiguousarray(Wkv[DIM:].T).astype(np.float16)      # [c, d]
    wp = np.ascontiguousarray(Wproj.T).astype(np.float16)          # [d, e]
    bias = np.ascontiguousarray(np.tile(bproj[None, :], (128, 1)))
    # epos[h, k, q] = exp(attn_pos[0, h, q, k])
    epos = np.ascontiguousarray(
        np.exp(attn_pos[0]).transpose(0, 2, 1)
    ).astype(np.float16)

    qT = np.ascontiguousarray(q.transpose(0, 2, 1)).astype(np.float16)    # [B, c, L]
    kvT = np.ascontiguousarray(kv.transpose(0, 2, 1)).astype(np.float16)  # [B, c, L]

    shared = {"wq": wq, "wk": wk, "wv": wv, "wp": wp, "bias": bias, "epos": epos}
    in_maps = []
    for b in range(B):
        m = dict(shared)
        m["qT"] = qT[b]
        m["kvT"] = kvT[b]
        in_maps.append(m)
    return in_maps


def kernel(q, kv, attn_pos, Wq, Wkv, Wproj, bproj):
    from concourse.bass_utils import run_bass_kernel_spmd

    nc = _get_nc()
    in_maps = _host_prep(q, kv, attn_pos, Wq, Wkv, Wproj, bproj)
    res = run_bass_kernel_spmd(nc, in_maps, list(range(NCORES)))
    return np.stack([res.results[b]["out"] for b in range(B)], axis=0)


# revision 13
# speedup vs baseline: 1.0367x; 1.0367x over previous
"""CrossAttention Trainium2 kernel — head-pair interleaved attention.

Full inputs in, full output out. Data-parallel over batch: core b computes
batch item b of 8.

Layouts as in v2a (everything transposed so the PE contraction dim is the
partition dim). Phase 2 processes HEAD PAIRS: the S matmuls for heads 2p
and 2p+1 alternate between PE row-halves (partitions 0-63 / 64-127), which
the PE array streams concurrently (~2x S throughput measured). The exp
activations on the scalar engine are the phase-2 floor; pt-multiplies are
split across vector and gpsimd, and half the output projection (d-chunks
0-2) is interleaved into pairs 3-5 to shorten the tail.

Rowsums are reshaped 1x1024 -> 4x256 via small DMAs so reciprocals use
more DVE lanes (1.7us instead of 6.5us per batch).
"""

import numpy as np

B, L, DIM, H, HD = 8, 1024, 768, 12, 64
NCORES = 8
CP = DIM // 128  # 6 chunks of the contraction/feature dim
KC = L // 128    # 8 k-chunks
SCALE = HD ** -0.5
LN_OFF = float(np.log(256.0))

_CACHE = {}


def _build():
    import concourse.bass as bass
    import concourse.mybir as mybir
    import concourse.tile as tile
    from concourse import bacc

    f32 = mybir.dt.float32
    f16 = mybir.dt.float16
    AF = mybir.ActivationFunctionType

    nc = bacc.Bacc("TRN2", target_bir_lowering=False, debug=False)

    qT = nc.dram_tensor("qT", [DIM, L], f16, kind="ExternalInput")
    kvT = nc.dram_tensor("kvT", [DIM, L], f16, kind="ExternalInput")
    wq = nc.dram_tensor("wq", [DIM, DIM], f16, kind="ExternalInput")  # [c, d]
    wk = nc.dram_tensor("wk", [DIM, DIM], f16, kind="ExternalInput")  # [c, d]
    wv = nc.dram_tensor("wv", [DIM, DIM], f16, kind="ExternalInput")  # [c, d]
    wp = nc.dram_tensor("wp", [DIM, DIM], f16, kind="ExternalInput")  # [d, e]
    bias = nc.dram_tensor("bias", [128, DIM], f32, kind="ExternalInput")
    epos = nc.dram_tensor("epos", [H, L, L], f16, kind="ExternalInput")  # [h,k,q]
    out = nc.dram_tensor("out", [L, DIM], f32, kind="ExternalOutput")
    rscr = nc.dram_tensor("rs_scratch", [72, 256], f32)

    # which (pair, k) slots run the first-half (d=0..2) out-projection for
    # which q-chunk; all after heads 0-5 are normalized at end of pair 2
    OPROJ_SLOTS = {
        (3, 2): 0, (3, 4): 1, (3, 6): 2,
        (4, 2): 3, (4, 4): 4, (4, 6): 5,
        (5, 2): 6, (5, 4): 7,
    }

    with tile.TileContext(nc) as tc:
        with tc.tile_pool(name="persist", bufs=1) as persist:
            QT = persist.tile([128, CP, L], f16)   # pair p: heads 2p, 2p+1
            KT = persist.tile([128, CP, L], f16)
            Vt = [
                persist.tile([128, H, HD + 1], f16, name=f"Vt{k}", tag=f"V{k}")
                for k in range(KC)
            ]
            wp_sb = persist.tile([128, CP, DIM], f16)
            bias_bc = persist.tile([128, DIM], f32)
            rs = persist.tile([72, 256], f32)      # 4 rows per head, 32-aligned batches
            recip = persist.tile([72, 256], f32)

            nc.sync.dma_start(wp_sb[:], wp.rearrange("(a p) d -> p a d", p=128))
            nc.sync.dma_start(bias_bc[:], bias[:])
            expb = persist.tile([128, 1], f32)
            nc.vector.memset(expb[:], -LN_OFF)

            # ---------------- phase 1: projections ----------------
            with (
                tc.tile_pool(name="ph1", bufs=1) as ph1,
                tc.tile_pool(name="psA", bufs=2, space="PSUM") as psA,
            ):
                q_c = [ph1.tile([128, L], f16, name=f"qc{c}") for c in range(CP)]
                kv_c = [ph1.tile([128, L], f16, name=f"kvc{c}") for c in range(CP)]
                wq_c = [ph1.tile([128, DIM], f16, name=f"wqc{c}") for c in range(CP)]
                wk_c = [ph1.tile([128, DIM], f16, name=f"wkc{c}") for c in range(CP)]
                wv_c = [ph1.tile([128, DIM], f16, name=f"wvc{c}") for c in range(CP)]
                for c in range(CP):
                    r = slice(c * 128, (c + 1) * 128)
                    nc.sync.dma_start(wq_c[c][:], wq[r, :])
                    nc.sync.dma_start(q_c[c][:], qT[r, :])
                for c in range(CP):
                    r = slice(c * 128, (c + 1) * 128)
                    nc.sync.dma_start(wk_c[c][:], wk[r, :])
                    nc.sync.dma_start(kv_c[c][:], kvT[r, :])
                for c in range(CP):
                    r = slice(c * 128, (c + 1) * 128)
                    nc.sync.dma_start(wv_c[c][:], wv[r, :])

                for w_c, x_c, dst in ((wq_c, q_c, QT), (wk_c, kv_c, KT)):
                    for pp in range(CP // 2):
                        psL = psA.tile([128, L], f32, tag="proj", name=f"psL{pp}")
                        psR = psA.tile([128, L], f32, tag="proj", name=f"psR{pp}")
                        for c in range(CP):
                            for pi, ps in ((0, psL), (1, psR)):
                                p = pp * 2 + pi
                                for hf in range(2):
                                    nc.tensor.matmul(
                                        ps[:, hf * 512:(hf + 1) * 512],
                                        w_c[c][:, p * 128:(p + 1) * 128],
                                        x_c[c][:, hf * 512:(hf + 1) * 512],
                                        start=(c == 0),
                                        stop=(c == CP - 1),
                                    )
                        nc.vector.tensor_copy(dst[:, pp * 2, :], psL[:])
                        nc.vector.tensor_copy(dst[:, pp * 2 + 1, :], psR[:])

                for k in range(KC):
                    ps = psA.tile([128, DIM], f32, tag="proj")
                    for c in range(CP):
                        for lo, sz in ((0, 512), (512, 256)):
                            nc.tensor.matmul(
                                ps[:, lo:lo + sz],
                                kv_c[c][:, k * 128:(k + 1) * 128],
                                wv_c[c][:, lo:lo + sz],
                                start=(c == 0),
                                stop=(c == CP - 1),
                            )
                    nc.vector.memset(Vt[k][:, :, HD:HD + 1], 1.0)
                    nc.scalar.copy(
                        Vt[k][:, :, 0:HD],
                        ps[:].rearrange("p (h d) -> p h d", d=HD),
                    )

            # ---------------- phase 2: attention (head pairs) -------------
            with (
                tc.tile_pool(name="xt", bufs=1) as xtp,
                tc.tile_pool(name="oaccp", bufs=8) as oaccp,
            ):
              with (
                tc.tile_pool(name="eposp", bufs=10) as eposp,
                tc.tile_pool(name="praw", bufs=4) as praw,
                tc.tile_pool(name="ptp", bufs=6) as ptp,
                tc.tile_pool(name="xtup", bufs=6) as xtup,
                tc.tile_pool(name="bcp", bufs=2) as bcp,
                tc.tile_pool(name="psS", bufs=2, space="PSUM") as psS,
                tc.tile_pool(name="psO", bufs=2, space="PSUM") as psO,
              ):
                XT = xtp.tile([128, CP, L], f16)
                xtu = [None] * H
                oacc = [None] * KC

                def rrow(h):
                    # 32-aligned batches: DVE ops must start at partition 0/32/64
                    if h < 6:
                        return h * 4
                    if h < 10:
                        return 32 + (h - 6) * 4
                    return 64 + (h - 10) * 4

                def gather_rowsum(h):
                    # xtu[h][64:65, :] (1x1024) -> rs 4x256 rows
                    r0 = rrow(h)
                    for s in range(4):
                        nc.sync.dma_start(
                            rs[r0 + s:r0 + s + 1, :],
                            xtu[h][64:65, s * 256:(s + 1) * 256],
                        )

                def normalize(h, eng):
                    p, sub = divmod(h, 2)
                    bc = bcp.tile([64, L], f32, name=f"bc{h}", tag="bc")
                    r0 = rrow(h)
                    for s in range(4):
                        nc.sync.dma_start(
                            bc[:, s * 256:(s + 1) * 256],
                            rscr[r0 + s:r0 + s + 1, :].broadcast_to([64, 256]),
                        )
                    eng.tensor_mul(
                        XT[sub * 64:(sub + 1) * 64, p, :],
                        xtu[h][0:64, :],
                        bc[:],
                    )

                for p in range(CP):
                    hA, hB = 2 * p, 2 * p + 1
                    o_psA = psO.tile([65, L], f32, tag="o", name=f"oA{p}")
                    o_psB = psO.tile([65, L], f32, tag="o", name=f"oB{p}")

                    def o_matmuls(k, ptA, ptB):
                        for hf in range(2):
                            hblk = slice(hf * 512, (hf + 1) * 512)
                            nc.tensor.matmul(
                                o_psA[:, hblk], Vt[k][:, hA, :], ptA[:, hblk],
                                start=(k == 0), stop=(k == KC - 1),
                            )
                            nc.tensor.matmul(
                                o_psB[:, hblk], Vt[k][:, hB, :], ptB[:, hblk],
                                start=(k == 0), stop=(k == KC - 1),
                            )

                    pend = None
                    for k in range(KC):
                        kblk = slice(k * 128, (k + 1) * 128)
                        s_psA = psS.tile([128, L], f32, tag="s", name=f"sA{p}_{k}")
                        s_psB = psS.tile([128, L], f32, tag="s", name=f"sB{p}_{k}")
                        for hf in range(2):
                            hblk = slice(hf * 512, (hf + 1) * 512)
                            nc.tensor.matmul(
                                s_psA[:, hblk], KT[0:64, p, kblk], QT[0:64, p, hblk],
                            )
                            nc.tensor.matmul(
                                s_psB[:, hblk], KT[64:128, p, kblk], QT[64:128, p, hblk],
                            )
                        prA = praw.tile([128, L], f16, tag="pr")
                        nc.scalar.activation(prA[:], s_psA[:], AF.Exp, bias=expb[:])
                        epA = eposp.tile([128, L], f16, tag="ep")
                        nc.sync.dma_start(epA[:], epos[hA, kblk, :])
                        ptA = ptp.tile([128, L], f16, tag="pt")
                        nc.vector.tensor_mul(ptA[:], prA[:], epA[:])

                        prB = praw.tile([128, L], f16, tag="pr")
                        nc.scalar.activation(prB[:], s_psB[:], AF.Exp, bias=expb[:])
                        epB = eposp.tile([128, L], f16, tag="ep")
                        nc.sync.dma_start(epB[:], epos[hB, kblk, :])
                        ptB = ptp.tile([128, L], f16, tag="pt")
                        if k in (0, 3, 6):
                            nc.gpsimd.tensor_mul(ptB[:], prB[:], epB[:])
                        else:
                            nc.vector.tensor_mul(ptB[:], prB[:], epB[:])

                        # O matmuls run one k-iteration behind so the slower
                        # pt producers never stall the tensor engine
                        if pend is not None:
                            o_matmuls(*pend)
                        pend = (k, ptA, ptB)

                        qc = OPROJ_SLOTS.get((p, k))
                        if qc is not None:
                            po = psS.tile([128, DIM], f32, tag="s", name=f"po{qc}")
                            for d in range(CP // 2):
                                for lo, sz in ((0, 512), (512, 256)):
                                    nc.tensor.matmul(
                                        po[:, lo:lo + sz],
                                        XT[:, d, qc * 128:(qc + 1) * 128],
                                        wp_sb[:, d, lo:lo + sz],
                                        start=(d == 0),
                                        stop=(d == CP // 2 - 1),
                                    )
                            oacc[qc] = oaccp.tile(
                                [128, DIM], f32, tag="oacc", name=f"oacc{qc}"
                            )
                            nc.vector.tensor_add(oacc[qc][:], po[:], bias_bc[:])

                    o_matmuls(*pend)

                    for h, o_ps in ((hA, o_psA), (hB, o_psB)):
                        xtu[h] = xtup.tile([65, L], f32, name=f"xtu{h}", tag="xtu")
                        nc.vector.tensor_copy(xtu[h][:], o_ps[:])
                        gather_rowsum(h)

                    if p == 2:
                        nc.vector.reciprocal(recip[0:24, :], rs[0:24, :])
                        nc.sync.dma_start(rscr[0:24, :], recip[0:24, :])
                        for hh in range(6):
                            normalize(hh, nc.vector if hh % 2 else nc.gpsimd)
                    if p == 4:
                        nc.vector.reciprocal(recip[32:48, :], rs[32:48, :])
                        nc.sync.dma_start(rscr[32:48, :], recip[32:48, :])
                        for hh in range(6, 10):
                            normalize(hh, nc.vector if hh % 2 else nc.gpsimd)
                    if p == 5:
                        nc.vector.reciprocal(recip[64:72, :], rs[64:72, :])
                        nc.sync.dma_start(rscr[64:72, :], recip[64:72, :])
                        for hh in range(10, 12):
                            normalize(hh, nc.vector if hh % 2 else nc.gpsimd)

              # ---------------- phase 3: second-half output projection ----
              with (
                  tc.tile_pool(name="outp", bufs=3) as outp,
                  tc.tile_pool(name="psOut", bufs=3, space="PSUM") as psOut,
              ):
                  for qc in range(KC):
                      ps = psOut.tile([128, DIM], f32)
                      for d in range(CP // 2, CP):
                          for lo, sz in ((0, 512), (512, 256)):
                              nc.tensor.matmul(
                                  ps[:, lo:lo + sz],
                                  XT[:, d, qc * 128:(qc + 1) * 128],
                                  wp_sb[:, d, lo:lo + sz],
                                  start=(d == CP // 2),
                                  stop=(d == CP - 1),
                              )
                      ot = outp.tile([128, DIM], f32)
                      nc.vector.tensor_add(ot[:], ps[:], oacc[qc][:])
                      nc.sync.dma_start(out[qc * 128:(qc + 1) * 128, :], ot[:])

    nc.compile()
    return nc


def _get_nc():
    if "nc" not in _CACHE:
        _CACHE["nc"] = _build()
    return _CACHE["nc"]


def _host_prep(q, kv, attn_pos, Wq, Wkv, Wproj, bproj):
    q = np.asarray(q, dtype=np.float32)
    kv = np.asarray(kv, dtype=np.float32)
    attn_pos = np.asarray(attn_pos, dtype=np.float32)
    Wq = np.asarray(Wq, dtype=np.float32)
    Wkv = np.asarray(Wkv, dtype=np.float32)
    Wproj = np.asarray(Wproj, dtype=np.float32)
    bproj = np.asarray(bproj, dtype=np.float32)

    wq = np.ascontiguousarray((Wq * SCALE).T).astype(np.float16)   # [c, d]
    wk = np.ascontiguousarray(Wkv[:DIM].T).astype(np.float16)      # [c, d]
    wv = np.ascontiguousarray(Wkv[DIM:].T).astype(np.float16)      # [c, d]
    wp = np.ascontiguousarray(Wproj.T).astype(np.float16)          # [d, e]
    bias = np.ascontiguousarray(np.tile(bproj[None, :], (128, 1)))
    # epos[h, k, q] = exp(attn_pos[0, h, q, k])
    epos = np.ascontiguousarray(
        np.exp(attn_pos[0]).transpose(0, 2, 1)
    ).astype(np.float16)

    qT = np.ascontiguousarray(q.transpose(0, 2, 1)).astype(np.float16)    # [B, c, L]
    kvT = np.ascontiguousarray(kv.transpose(0, 2, 1)).astype(np.float16)  # [B, c, L]

    shared = {"wq": wq, "wk": wk, "wv": wv, "wp": wp, "bias": bias, "epos": epos}
    in_maps = []
    for b in range(B):
        m = dict(shared)
        m["qT"] = qT[b]
        m["kvT"] = kvT[b]
        in_maps.append(m)
    return in_maps


def kernel(q, kv, attn_pos, Wq, Wkv, Wproj, bproj):
    from concourse.bass_utils import run_bass_kernel_spmd

    nc = _get_nc()
    in_maps = _host_prep(q, kv, attn_pos, Wq, Wkv, Wproj, bproj)
    res = run_bass_kernel_spmd(nc, in_maps, list(range(NCORES)))
    return np.stack([res.results[b]["out"] for b in range(B)], axis=0)
